# revision 1
# baseline (speedup 1.0000x reference)
"""Trainium2 Bass kernel for quantized-linear + LoRA (nn_LoRALinear).

Computes, for x:(4,2048,4096) f32, weight_quant:(4096,4096) i32 in [0,16),
scale/zero:(4096,1) f32, lora_A:(16,4096), lora_B:(4096,16), bias:(4096,):

    W = (weight_quant - zero) * scale
    y = x @ W.T + bias + 2.0 * (x @ lora_A.T) @ lora_B.T

Sharding across 8 NeuronCores: 4-way over tokens x 2-way over out-features.
Per core: x-slice (2048, 4096), weight rows slice (2048 of 4096), output
block (2048 tokens, 2048 features); host only slices inputs / stitches blocks.

Device algorithm (per core):

    P[o,n]   = sum_d (wq[o,d]-8) * x[n,d]          (PE; fp8e4 weights (exact
                                                    ints) x bf16 moving)
             + sum_r B2[o,r] * t[r,n]              (K=17 fp32r matmul into the
             + (8-zero[o]) * rowsum[n]              same psum accumulation)
    y[n,o]   = scale[o] * P[o,n] + bias[o]         (ScalarE psum eviction)

with t = lora_A @ x.T augmented by a ones-row giving rowsum, B2 = 2*lora_B/
scale. Output lands transposed [o,n]; PE de-transposes before DMA-out.
"""
import os
import sys
import types

sys.path.insert(0, "/opt/trn_rl_repo")

import numpy as np

import concourse.bass as bass
import concourse.mybir as mybir
import concourse.tile as tile
from concourse import bacc
from concourse.bass_utils import run_bass_kernel_spmd
from concourse.masks import make_identity

F32 = mybir.dt.float32
F32R = mybir.dt.float32r
BF16 = mybir.dt.bfloat16
FP8 = mybir.dt.float8e4
I32 = mybir.dt.int32

# Problem shape (hardcoded per contract)
B, S, D, O, R = 4, 2048, 4096, 4096, 16
SCALING = 32.0 / 16.0
N_TOK = B * S            # 8192 tokens
T_SH, F_SH = 4, 2        # token shards x feature shards = 8 cores
N_SH = N_TOK // T_SH     # 2048 tokens per core
O_SH = O // F_SH         # 2048 out-features per core

NT = 4                   # n tiles per core
N_TILE = N_SH // NT      # 512
KC = D // 128            # 32 contraction chunks
OT = O_SH // 128         # 16 o tiles
OQ = 4                   # o tiles per psum pass
WQ_CENTER = 8.0          # center wq (exact in fp8e4; smaller dot magnitude)


def _ensure_ntff_hook():
    """Best-effort: register the axon NTFF profile hook so trace=True works."""
    try:
        import antenv
        if "antenv.axon_hooks" not in sys.modules:
            hooks_mod = types.ModuleType("antenv.axon_hooks")
            hooks_mod._hook = None
            hooks_mod.set_axon_ntff_profile_hook = lambda h: setattr(hooks_mod, "_hook", h)
            hooks_mod.get_axon_ntff_profile_hook = lambda: hooks_mod._hook
            sys.modules["antenv.axon_hooks"] = hooks_mod
            antenv.axon_hooks = hooks_mod
        from trn_agent_boot.trn_boot import _ntff_profile_via_ctypes
        sys.modules["antenv.axon_hooks"].set_axon_ntff_profile_hook(
            _ntff_profile_via_ctypes("/opt/axon/libaxon_pjrt.so")
        )
        import concourse.bass_utils as bu
        bu.upload_artifacts = lambda tmpdir: tmpdir
    except Exception:
        pass


def build_nc() -> bass.Bass:
    nc = bacc.Bacc("TRN2", target_bir_lowering=False, debug=False)

    x_d = nc.dram_tensor("x", (N_SH, D), F32, kind="ExternalInput")
    wq_d = nc.dram_tensor("wq", (O_SH, D), I32, kind="ExternalInput")
    scale_d = nc.dram_tensor("scale", (O_SH,), F32, kind="ExternalInput")
    zero_d = nc.dram_tensor("zero", (O_SH,), F32, kind="ExternalInput")
    bias_d = nc.dram_tensor("bias", (O_SH,), F32, kind="ExternalInput")
    a_d = nc.dram_tensor("lora_a", (R, D), F32, kind="ExternalInput")
    b_d = nc.dram_tensor("lora_b", (O_SH, R), F32, kind="ExternalInput")
    y_d = nc.dram_tensor("y", (N_SH, O_SH), F32, kind="ExternalOutput")

    with tile.TileContext(nc) as tc:
        with (
            tc.tile_pool(name="const", bufs=1) as cpool,
            tc.tile_pool(name="wt", bufs=1) as wtpool,
            tc.tile_pool(name="xt", bufs=2) as xtpool,
            tc.tile_pool(name="stage", bufs=3) as stage,
            tc.tile_pool(name="cvt", bufs=2) as cvt,
            tc.tile_pool(name="outp", bufs=3) as outp,
            tc.tile_pool(name="dram", bufs=1, space="DRAM") as dpool,
            tc.tile_pool(name="ps_small", bufs=3, space="PSUM") as ps_small,
            tc.tile_pool(name="ps_t", bufs=1, space="PSUM") as ps_tp,
            tc.tile_pool(name="ps_acc", bufs=4, space="PSUM") as ps_accp,
        ):
            # ---------------- constants ----------------
            ident_b = cpool.tile([128, 128], BF16)
            make_identity(nc, ident_b)
            ident_f = cpool.tile([128, 128], F32)
            make_identity(nc, ident_f)
            ident_r = cpool.tile([128, 128], F32R)
            nc.vector.tensor_copy(ident_r[:], ident_f[:])

            # scale/bias/zero as [128 partitions, 16 o-tiles] f32
            scale_sb = cpool.tile([128, OT], F32)
            bias_sb = cpool.tile([128, OT], F32)
            zero_sb = cpool.tile([128, OT], F32)
            nc.sync.dma_start(scale_sb[:], scale_d.rearrange("(t p) -> p t", p=128))
            nc.sync.dma_start(bias_sb[:], bias_d.rearrange("(t p) -> p t", p=128))
            nc.sync.dma_start(zero_sb[:], zero_d.rearrange("(t p) -> p t", p=128))
            rcp_sb = cpool.tile([128, OT], F32)
            nc.vector.reciprocal(rcp_sb[:], scale_sb[:])
            rcp2_sb = cpool.tile([128, OT], F32)
            nc.vector.tensor_scalar_mul(rcp2_sb[:], rcp_sb[:], float(SCALING))

            # B2augT [18, OT, 128] fp32r: rows 0..15 = (2*B/scale).T,
            # row 16 = (WQ_CENTER - zero)  [pairs with rowsum row of t_aug],
            # row 17 = bias/scale          [pairs with the ones row of t_sb]
            b2augT = cpool.tile([18, OT, 128], F32R)
            for t in range(OT):
                bblk = stage.tile([128, R], F32, tag="bblk")
                nc.sync.dma_start(bblk[:], b_d[t * 128:(t + 1) * 128, :])
                pre = stage.tile([128, 18], F32R, tag="pre")
                nc.vector.tensor_scalar(
                    out=pre[:, 0:R], in0=bblk[:],
                    scalar1=rcp2_sb[:, t:t + 1], scalar2=None,
                    op0=mybir.AluOpType.mult,
                )
                nc.vector.tensor_scalar(
                    out=pre[:, R:R + 1], in0=zero_sb[:, t:t + 1],
                    scalar1=-1.0, scalar2=float(WQ_CENTER),
                    op0=mybir.AluOpType.mult, op1=mybir.AluOpType.add,
                )
                nc.vector.tensor_mul(
                    pre[:, R + 1:R + 2], bias_sb[:, t:t + 1], rcp_sb[:, t:t + 1]
                )
                psb = ps_small.tile([18, 128], F32R, tag="ps_sm")
                nc.tensor.transpose(psb[:], pre[:], ident_r[:])
                nc.vector.tensor_copy(b2augT[:, t, :], psb[:].bitcast(F32))

            # A_augT [128, KC, 17] bf16: cols 0..15 = A.T chunk, col16 = ones
            a_augT = cpool.tile([128, KC, R + 1], BF16)
            nc.gpsimd.memset(a_augT[:, :, R:R + 1], 1.0)
            ones32 = cpool.tile([32, N_TILE], F32)
            nc.gpsimd.memset(ones32[:], 1.0)
            for k in range(KC):
                a_st = stage.tile([R, 128], F32, tag="a_st")
                nc.sync.dma_start(a_st[:], a_d[:, k * 128:(k + 1) * 128])
                a_bf = cvt.tile([R, 128], BF16, tag="a_bf")
                nc.vector.tensor_copy(a_bf[:], a_st[:])
                psa = ps_small.tile([128, R], BF16, tag="ps_sm")
                nc.tensor.transpose(psa[:], a_bf[:], ident_b[0:R, 0:R])
                nc.vector.tensor_copy(a_augT[:, k, 0:R], psa[:])

            # x is cast-DMA'd to bf16 DRAM scratch then transposed by the DMA
            # xbar; wq is cast-DMA'd to SBUF and transposed on the (early-idle)
            # PE, with the -8 centering + fp8 narrowing in the DVE eviction.
            x_bf_s = dpool.tile([N_SH, D], BF16)

            def emit_x_cast(nt):
                for g in range(N_TILE // 128):
                    r0 = nt * N_TILE + g * 128
                    xc = cvt.tile([128, D], BF16, tag="xcast")
                    nc.gpsimd.dma_start(xc[:], x_d[r0:r0 + 128, :])
                    nc.sync.dma_start(x_bf_s[r0:r0 + 128, :], xc[:])

            # ------- Wt: transposed centered weights, fp8e4 (exact), resident -------
            # wt_og[og][p=d_in, k, oi, o_in] = wq[(og*4+oi)*128+o_in, k*128+p] - 8
            # Split into OQ separate tiles; builds are emitted interleaved with
            # the first n-tile's compute so the PE never queues idle behind them.
            wt_og = []
            for og in range(OQ):
                wt_g_tile = wtpool.tile([128, KC, OQ, 128], FP8, tag=f"wt{og}")
                wt_og.append(wt_g_tile)

            def emit_og_build(og):
                wt_g = wt_og[og]
                for rg in range(4):
                    wqc = cvt.tile([128, D], BF16, tag="wqcast")
                    nc.gpsimd.dma_start(
                        wqc[:], wq_d[og * 512 + rg * 128: og * 512 + (rg + 1) * 128, :]
                    )
                    for k in range(KC):
                        pst = ps_small.tile([128, 128], BF16, tag="ps_sm")
                        nc.tensor.transpose(
                            pst[:], wqc[:, k * 128:(k + 1) * 128], ident_b[:]
                        )
                        # center by -8 during the psum eviction (bf16 -> fp8)
                        nc.vector.tensor_scalar(
                            out=wt_g[:, k, rg, :], in0=pst[:],
                            scalar1=-WQ_CENTER, scalar2=None,
                            op0=mybir.AluOpType.add,
                        )

            # ---------------- main loop ----------------
            def emit_nt_prep(nt):
                # xT bf16 [128, KC, N_TILE] via one xbar DMA-transpose
                xT = xtpool.tile([128, KC, N_TILE], BF16, tag="xT")
                nc.sync.dma_start_transpose(
                    xT[:], x_bf_s[nt * N_TILE:(nt + 1) * N_TILE, :]
                )
                # t_aug [17, N_TILE] psum: rows 0..15 = A@x.T, row16 = rowsum
                ps_t = ps_tp.tile([R + 1, N_TILE], F32)
                for k in range(KC):
                    nc.tensor.matmul(
                        ps_t[:], a_augT[:, k, :], xT[:, k, :],
                        start=(k == 0), stop=(k == KC - 1),
                    )
                # t_sb rows 0..16 = t_aug, row 17 = 1.0 (ones base, overwrite)
                t_sb = outp.tile([32, N_TILE], F32R, tag="t_sb")
                nc.vector.tensor_copy(t_sb[:], ones32[:])
                nc.vector.tensor_copy(t_sb[0:R + 1, :], ps_t[:])
                return xT, t_sb

            def emit_nt_oq(nt, oq, xT, t_sb):
                accs = []
                for _oi in range(OQ):
                    acc_tile = ps_accp.tile([128, N_TILE], F32, tag="acc")
                    accs.append(acc_tile)
                for k in range(KC):
                    for oi in range(OQ):
                        nc.tensor.matmul(
                            accs[oi][:], wt_og[oq][:, k, oi, :], xT[:, k, :],
                            start=(k == 0), stop=False,
                        )
                for oi in range(OQ):
                    ot = oq * OQ + oi
                    # lora + zero-correction + bias: K=18 fp32r matmul
                    nc.tensor.matmul(
                        accs[oi][:], b2augT[:, ot, :], t_sb[0:18, :],
                        start=False, stop=True,
                    )
                    # yT tile = scale[o]*P  (bias folded into the K=18 matmul)
                    yT_sb = outp.tile([128, N_TILE], F32, tag="yT")
                    nc.scalar.activation(
                        yT_sb[:], accs[oi][:],
                        mybir.ActivationFunctionType.Copy,
                        scale=scale_sb[:, ot:ot + 1],
                    )
                    # de-transpose [o,n] -> [n,o]; store
                    yst = outp.tile([128, N_TILE // 128, 128], F32, tag="yst")
                    for sub in range(N_TILE // 128):
                        psd = ps_small.tile([128, 128], F32, tag="ps_sm")
                        nc.tensor.transpose(
                            psd[:], yT_sb[:, sub * 128:(sub + 1) * 128],
                            ident_f[:],
                        )
                        nc.vector.tensor_copy(yst[:, sub, :], psd[:])
                    nc.sync.dma_start(
                        y_d[nt * N_TILE:(nt + 1) * N_TILE,
                            ot * 128:(ot + 1) * 128]
                        .rearrange("(s p) f -> p s f", p=128),
                        yst[:],
                    )

            # interleaved emission: casts/builds slotted between the first
            # n-tile's compute phases so neither PE nor DMA queues stall
            emit_x_cast(0)
            emit_og_build(0)
            xT0, t_sb0 = emit_nt_prep(0)
            emit_nt_oq(0, 0, xT0, t_sb0)
            emit_og_build(1)
            emit_nt_oq(0, 1, xT0, t_sb0)
            emit_og_build(2)
            emit_nt_oq(0, 2, xT0, t_sb0)
            emit_og_build(3)
            emit_x_cast(1)
            emit_nt_oq(0, 3, xT0, t_sb0)
            for nt in range(1, NT):
                xT, t_sb = emit_nt_prep(nt)
                emit_nt_oq(nt, 0, xT, t_sb)
                if nt + 1 < NT:
                    emit_x_cast(nt + 1)
                for oq in range(1, OQ):
                    emit_nt_oq(nt, oq, xT, t_sb)

    nc.finalize()
    return nc


_NC_CACHE: dict = {}


def _get_nc() -> bass.Bass:
    if "nc" not in _NC_CACHE:
        _ensure_ntff_hook()
        _NC_CACHE["nc"] = build_nc()
    return _NC_CACHE["nc"]


def kernel(x, weight_quant, scale, zero, lora_A, lora_B, bias):
    x = np.ascontiguousarray(np.asarray(x, dtype=np.float32)).reshape(N_TOK, D)
    weight_quant = np.asarray(weight_quant, dtype=np.int32)
    scale_f = np.asarray(scale, dtype=np.float32).reshape(O)
    zero_f = np.asarray(zero, dtype=np.float32).reshape(O)
    bias_f = np.asarray(bias, dtype=np.float32).reshape(O)
    lora_A = np.ascontiguousarray(np.asarray(lora_A, dtype=np.float32))
    lora_B = np.ascontiguousarray(np.asarray(lora_B, dtype=np.float32))

    nc = _get_nc()

    in_maps = []
    for core in range(T_SH * F_SH):
        ti, fi = core % T_SH, core // T_SH
        osl = slice(fi * O_SH, (fi + 1) * O_SH)
        in_maps.append({
            "x": np.ascontiguousarray(x[ti * N_SH:(ti + 1) * N_SH]),
            "wq": np.ascontiguousarray(weight_quant[osl]),
            "scale": np.ascontiguousarray(scale_f[osl]),
            "zero": np.ascontiguousarray(zero_f[osl]),
            "bias": np.ascontiguousarray(bias_f[osl]),
            "lora_a": lora_A,
            "lora_b": np.ascontiguousarray(lora_B[osl]),
        })

    trace = bool(os.environ.get("BASS_KERNEL_TRACE"))
    res = run_bass_kernel_spmd(
        nc, in_maps, core_ids=list(range(T_SH * F_SH)), trace=trace,
    )
    if trace:
        _NC_CACHE["last_exec_time_ns"] = res.exec_time_ns
        _NC_CACHE["last_results"] = res

    y = np.empty((N_TOK, O), dtype=np.float32)
    for core in range(T_SH * F_SH):
        ti, fi = core % T_SH, core // T_SH
        y[ti * N_SH:(ti + 1) * N_SH, fi * O_SH:(fi + 1) * O_SH] = \
            res.results[core]["y"]
    return y.reshape(B, S, O)



# revision 10
# speedup vs baseline: 1.7558x; 1.7558x over previous
"""Trainium2 Bass kernel for quantized-linear + LoRA (nn_LoRALinear).

Computes, for x:(4,2048,4096) f32, weight_quant:(4096,4096) i32 in [0,16),
scale/zero:(4096,1) f32, lora_A:(16,4096), lora_B:(4096,16), bias:(4096,):

    W = (weight_quant - zero) * scale
    y = x @ W.T + bias + 2.0 * (x @ lora_A.T) @ lora_B.T

Sharding across 8 NeuronCores: 4-way over tokens x 2-way over out-features.
Per core: x-slice (2048, 4096), weight rows slice (2048 of 4096), output
block (2048 tokens, 2048 features); host slices/permutes inputs and
stitches/transposes output blocks (layout only, no arithmetic).

Device algorithm (per core):

    P[o,n]   = sum_d (wq[o,d]-8) * x[n,d]          (PE; fp8e4 DoubleRow:
                                                    both operands fp8,
                                                    2 k-chunks per matmul)
             + sum_r B2[o,r] * t[r,n]              (K=18 fp32r matmul into the
             + (8-zero[o]) * rowsum[n]              same psum accumulation)
    y[o,n]   = scale[o] * P[o,n] + bias[o]         (ScalarE psum eviction)

with t = lora_A @ x.T (bf16) augmented by a ones-row giving rowsum,
B2 = 2*lora_B/scale. Weights arrive host-pre-transposed as uint8 [d, o]
(values 0..15) and are centered to fp8 on the DVE; x is cast-DMA'd to a
bf16 DRAM scratch, DMA-transposed to xT, and DVE-narrowed to fp8.
Output stays transposed [o, n]; the host de-transposes while stitching.
"""
import os
import sys
import types

sys.path.insert(0, "/opt/trn_rl_repo")

import numpy as np

import concourse.bass as bass
import concourse.mybir as mybir
import concourse.tile as tile
from concourse import bacc
from concourse.bass_utils import run_bass_kernel_spmd
from concourse.masks import make_identity

F32 = mybir.dt.float32
F32R = mybir.dt.float32r
BF16 = mybir.dt.bfloat16
FP8 = mybir.dt.float8e4
U8 = mybir.dt.uint8

DR = mybir.MatmulPerfMode.DoubleRow

# Problem shape (hardcoded per contract)
B, S, D, O, R = 4, 2048, 4096, 4096, 16
SCALING = 32.0 / 16.0
N_TOK = B * S            # 8192 tokens
T_SH, F_SH = 4, 2        # token shards x feature shards = 8 cores
N_SH = N_TOK // T_SH     # 2048 tokens per core
O_SH = O // F_SH         # 2048 out-features per core

NT = 4                   # n tiles per core
N_TILE = N_SH // NT      # 512
KC = D // 128            # 32 contraction chunks of 128
KD = KC // 2             # 16 DoubleRow chunk-pairs
OT = O_SH // 128         # 16 o tiles
OQ = 4                   # o tiles per psum pass
WQ_CENTER = 8.0          # center wq (exact in fp8e4; small dot magnitude)


def _ensure_ntff_hook():
    """Best-effort: register the axon NTFF profile hook so trace=True works."""
    try:
        import antenv
        if "antenv.axon_hooks" not in sys.modules:
            hooks_mod = types.ModuleType("antenv.axon_hooks")
            hooks_mod._hook = None
            hooks_mod.set_axon_ntff_profile_hook = lambda h: setattr(hooks_mod, "_hook", h)
            hooks_mod.get_axon_ntff_profile_hook = lambda: hooks_mod._hook
            sys.modules["antenv.axon_hooks"] = hooks_mod
            antenv.axon_hooks = hooks_mod
        from trn_agent_boot.trn_boot import _ntff_profile_via_ctypes
        sys.modules["antenv.axon_hooks"].set_axon_ntff_profile_hook(
            _ntff_profile_via_ctypes("/opt/axon/libaxon_pjrt.so")
        )
        import concourse.bass_utils as bu
        bu.upload_artifacts = lambda tmpdir: tmpdir
    except Exception:
        pass


DEBUG = bool(os.environ.get("BASS_KERNEL_DEBUG"))


def build_nc() -> bass.Bass:
    nc = bacc.Bacc("TRN2", target_bir_lowering=False, debug=False)

    x_d = nc.dram_tensor("x", (N_SH, D), F32, kind="ExternalInput")
    wqt_d = nc.dram_tensor("wqt", (D, O_SH), U8, kind="ExternalInput")
    scale_d = nc.dram_tensor("scale", (O_SH,), F32, kind="ExternalInput")
    zero_d = nc.dram_tensor("zero", (O_SH,), F32, kind="ExternalInput")
    bias_d = nc.dram_tensor("bias", (O_SH,), F32, kind="ExternalInput")
    a_d = nc.dram_tensor("lora_at", (D, R), F32, kind="ExternalInput")
    b_d = nc.dram_tensor("lora_b", (O_SH, R), F32, kind="ExternalInput")
    y_d = nc.dram_tensor("y", (O_SH, N_SH), F32, kind="ExternalOutput")
    if DEBUG:
        dbg_wt_d = nc.dram_tensor("dbg_wt", (128, KC, OT, 128), FP8,
                                  kind="ExternalOutput")
        dbg_xt8_d = nc.dram_tensor("dbg_xt8", (128, KC, N_TILE), FP8,
                                   kind="ExternalOutput")
        dbg_aaug_d = nc.dram_tensor("dbg_aaug", (128, KC, R + 1), BF16,
                                    kind="ExternalOutput")
        dbg_tsb_d = nc.dram_tensor("dbg_tsb", (32, N_TILE), F32,
                                   kind="ExternalOutput")

    with tile.TileContext(nc) as tc:
        with (
            tc.tile_pool(name="const", bufs=1) as cpool,
            tc.tile_pool(name="wt", bufs=1) as wtpool,
            tc.tile_pool(name="xt", bufs=1) as xtpool,
            tc.tile_pool(name="xt8", bufs=2) as xt8pool,
            tc.tile_pool(name="stage", bufs=3) as stage,
            tc.tile_pool(name="cvt", bufs=2) as cvt,
            tc.tile_pool(name="tsb", bufs=2) as tsbpool,
            tc.tile_pool(name="outp", bufs=3) as outp,
            tc.tile_pool(name="dram", bufs=1, space="DRAM") as dpool,
            tc.tile_pool(name="ps_small", bufs=2, space="PSUM") as ps_small,
            tc.tile_pool(name="ps_t", bufs=1, space="PSUM") as ps_tp,
            tc.tile_pool(name="ps_acc", bufs=4, space="PSUM") as ps_accp,
        ):
            # ---------------- constants ----------------
            ident_f = cpool.tile([128, 128], F32)
            make_identity(nc, ident_f)
            ident_r = cpool.tile([128, 128], F32R)
            nc.vector.tensor_copy(ident_r[:], ident_f[:])

            # scale/bias/zero as [128 partitions, 16 o-tiles] f32
            scale_sb = cpool.tile([128, OT], F32)
            bias_sb = cpool.tile([128, OT], F32)
            zero_sb = cpool.tile([128, OT], F32)
            nc.sync.dma_start(scale_sb[:], scale_d.rearrange("(t p) -> p t", p=128))
            nc.sync.dma_start(bias_sb[:], bias_d.rearrange("(t p) -> p t", p=128))
            nc.sync.dma_start(zero_sb[:], zero_d.rearrange("(t p) -> p t", p=128))
            rcp_sb = cpool.tile([128, OT], F32)
            nc.vector.reciprocal(rcp_sb[:], scale_sb[:])
            rcp2_sb = cpool.tile([128, OT], F32)
            nc.vector.tensor_scalar_mul(rcp2_sb[:], rcp_sb[:], float(SCALING))

            # B2augT [18, OT, 128] fp32r: rows 0..15 = (2*B/scale).T,
            # row 16 = (WQ_CENTER - zero)  [pairs with rowsum row of t_aug],
            # row 17 = bias/scale          [pairs with the ones row of t_sb]
            b2augT = cpool.tile([18, OT, 128], F32R)
            for t in range(OT):
                bblk = stage.tile([128, R], F32, tag="bblk")
                nc.sync.dma_start(bblk[:], b_d[t * 128:(t + 1) * 128, :])
                pre = stage.tile([128, 18], F32R, tag="pre")
                nc.vector.tensor_scalar(
                    out=pre[:, 0:R], in0=bblk[:],
                    scalar1=rcp2_sb[:, t:t + 1], scalar2=None,
                    op0=mybir.AluOpType.mult,
                )
                nc.vector.tensor_scalar(
                    out=pre[:, R:R + 1], in0=zero_sb[:, t:t + 1],
                    scalar1=-1.0, scalar2=float(WQ_CENTER),
                    op0=mybir.AluOpType.mult, op1=mybir.AluOpType.add,
                )
                nc.vector.tensor_mul(
                    pre[:, R + 1:R + 2], bias_sb[:, t:t + 1], rcp_sb[:, t:t + 1]
                )
                psb = ps_small.tile([18, 128], F32R, tag="ps_sm")
                nc.tensor.transpose(psb[:], pre[:], ident_r[:])
                nc.vector.tensor_copy(b2augT[:, t, :], psb[:].bitcast(F32))

            # A_augT [128, KC, 17] bf16: cols 0..15 = A.T chunk, col16 = ones.
            # A.T arrives host-pre-transposed; one 3D cast-DMA to a
            # contiguous staging tile, then a DVE copy into the aug layout.
            a_augT = cpool.tile([128, KC, R + 1], BF16)
            nc.gpsimd.memset(a_augT[:, :, R:R + 1], 1.0)
            ones32 = cpool.tile([32, N_TILE], F32)
            nc.gpsimd.memset(ones32[:], 1.0)
            a_stg = cpool.tile([128, KC, R], BF16)
            nc.gpsimd.dma_start(
                a_stg[:], a_d.rearrange("(k p) r -> p k r", p=128)
            )
            nc.vector.tensor_copy(a_augT[:, :, 0:R], a_stg[:])

            # x is cast-DMA'd to bf16 DRAM scratch then transposed by the DMA
            # xbar into xT; DVE narrows xT to fp8 for the DoubleRow matmuls.
            x_bf_s = dpool.tile([N_SH, D], BF16)

            def emit_x_cast(nt):
                for g in range(N_TILE // 128):
                    r0 = nt * N_TILE + g * 128
                    xc = cvt.tile([128, D], BF16, tag="xcast")
                    nc.gpsimd.dma_start(xc[:], x_d[r0:r0 + 128, :])
                    nc.sync.dma_start(x_bf_s[r0:r0 + 128, :], xc[:])

            # ------- Wt: transposed centered weights, fp8e4 (exact), resident -------
            # wt8[p, k, ot, oi] = wqt[k*128+p, ot*128+oi] - 8
            # Host supplies wqt = wq.T as uint8; cast-DMA to bf16 staging,
            # DVE applies the -8 centering while narrowing to fp8.
            wt8 = wtpool.tile([128, KC, OT, 128], FP8)

            def emit_og_build(og):
                # o-column group og covers o in [og*512, (og+1)*512)
                for k4 in range(KC // 4):
                    wst = stage.tile([128, 4, 512], BF16, tag="wst")
                    nc.gpsimd.dma_start(
                        wst[:],
                        wqt_d[k4 * 512:(k4 + 1) * 512,
                              og * 512:(og + 1) * 512]
                        .rearrange("(kk p) o -> p kk o", p=128),
                    )
                    nc.vector.tensor_scalar(
                        out=wt8[:, k4 * 4:(k4 + 1) * 4,
                                og * OQ:(og + 1) * OQ, :],
                        in0=wst[:],
                        scalar1=-WQ_CENTER, scalar2=None,
                        op0=mybir.AluOpType.add,
                    )

            # ---------------- main loop ----------------
            def emit_nt_prep(nt):
                # xT bf16 [128, KC, N_TILE] via one xbar DMA-transpose
                xT = xtpool.tile([128, KC, N_TILE], BF16, tag="xT")
                nc.sync.dma_start_transpose(
                    xT[:], x_bf_s[nt * N_TILE:(nt + 1) * N_TILE, :]
                )
                # fp8 copy for the DoubleRow matmuls
                xT8 = xt8pool.tile([128, KC, N_TILE], FP8, tag="xT8")
                nc.vector.tensor_copy(xT8[:], xT[:])
                # t_aug [17, N_TILE] psum: rows 0..15 = A@x.T, row16 = rowsum
                ps_t = ps_tp.tile([R + 1, N_TILE], F32)
                for k in range(KC):
                    nc.tensor.matmul(
                        ps_t[:], a_augT[:, k, :], xT[:, k, :],
                        start=(k == 0), stop=(k == KC - 1),
                    )
                # t_sb rows 0..16 = t_aug, row 17 = 1.0 (ones base, overwrite)
                t_sb = tsbpool.tile([32, N_TILE], F32R, tag="t_sb")
                nc.vector.tensor_copy(t_sb[:], ones32[:])
                nc.vector.tensor_copy(t_sb[0:R + 1, :], ps_t[:])
                return xT8, t_sb

            def emit_nt_oq(nt, oq, xT8, t_sb):
                accs = []
                for _oi in range(OQ):
                    acc_tile = ps_accp.tile([128, N_TILE], F32, tag="acc")
                    accs.append(acc_tile)
                for kd in range(KD):
                    for oi in range(OQ):
                        nc.tensor.matmul(
                            accs[oi][:],
                            wt8[:, 2 * kd:2 * kd + 2, oq * OQ + oi, :],
                            xT8[:, 2 * kd:2 * kd + 2, :],
                            start=(kd == 0), stop=False,
                            perf_mode=DR,
                        )
                for oi in range(OQ):
                    ot = oq * OQ + oi
                    # lora + zero-correction + bias: K=18 fp32r matmul
                    nc.tensor.matmul(
                        accs[oi][:], b2augT[:, ot, :], t_sb[0:18, :],
                        start=False, stop=True,
                    )
                    # yT tile = scale[o]*P  (bias folded into the K=18 matmul)
                    yT_sb = outp.tile([128, N_TILE], F32, tag="yT")
                    nc.scalar.activation(
                        yT_sb[:], accs[oi][:],
                        mybir.ActivationFunctionType.Copy,
                        scale=scale_sb[:, ot:ot + 1],
                    )
                    # store transposed [o, n]; host de-transposes
                    nc.sync.dma_start(
                        y_d[ot * 128:(ot + 1) * 128,
                            nt * N_TILE:(nt + 1) * N_TILE],
                        yT_sb[:],
                    )

            # interleaved emission: casts/builds slotted between the first
            # n-tile's compute phases so neither PE nor DMA queues stall
            emit_x_cast(0)
            emit_og_build(0)
            xT8_0, t_sb0 = emit_nt_prep(0)
            if DEBUG:
                nc.sync.dma_start(dbg_xt8_d[:], xT8_0[:])
                nc.sync.dma_start(dbg_aaug_d[:], a_augT[:])
                nc.sync.dma_start(dbg_tsb_d[:], t_sb0[:].bitcast(F32))
            emit_nt_oq(0, 0, xT8_0, t_sb0)
            emit_og_build(1)
            emit_nt_oq(0, 1, xT8_0, t_sb0)
            emit_og_build(2)
            emit_nt_oq(0, 2, xT8_0, t_sb0)
            emit_og_build(3)
            emit_x_cast(1)
            emit_nt_oq(0, 3, xT8_0, t_sb0)
            if DEBUG:
                nc.sync.dma_start(dbg_wt_d[:], wt8[:])
            for nt in range(1, NT):
                xT8, t_sb = emit_nt_prep(nt)
                emit_nt_oq(nt, 0, xT8, t_sb)
                if nt + 1 < NT:
                    emit_x_cast(nt + 1)
                for oq in range(1, OQ):
                    emit_nt_oq(nt, oq, xT8, t_sb)

    nc.finalize()
    return nc


_NC_CACHE: dict = {}


def _get_nc() -> bass.Bass:
    if "nc" not in _NC_CACHE:
        _ensure_ntff_hook()
        _NC_CACHE["nc"] = build_nc()
    return _NC_CACHE["nc"]


def kernel(x, weight_quant, scale, zero, lora_A, lora_B, bias):
    x = np.ascontiguousarray(np.asarray(x, dtype=np.float32)).reshape(N_TOK, D)
    weight_quant = np.asarray(weight_quant, dtype=np.int32)
    scale_f = np.asarray(scale, dtype=np.float32).reshape(O)
    zero_f = np.asarray(zero, dtype=np.float32).reshape(O)
    bias_f = np.asarray(bias, dtype=np.float32).reshape(O)
    lora_A = np.ascontiguousarray(np.asarray(lora_A, dtype=np.float32))
    lora_B = np.ascontiguousarray(np.asarray(lora_B, dtype=np.float32))

    nc = _get_nc()

    # host-side layout marshaling: transposed lora_A and transposed uint8
    # weight blocks per feature shard (values 0..15 — lossless repack)
    lora_at = np.ascontiguousarray(lora_A.T)
    wqt_by_f = []
    for fi in range(F_SH):
        osl = slice(fi * O_SH, (fi + 1) * O_SH)
        wqt_by_f.append(
            np.ascontiguousarray(weight_quant[osl].T.astype(np.uint8))
        )

    in_maps = []
    for core in range(T_SH * F_SH):
        ti, fi = core % T_SH, core // T_SH
        osl = slice(fi * O_SH, (fi + 1) * O_SH)
        in_maps.append({
            "x": np.ascontiguousarray(x[ti * N_SH:(ti + 1) * N_SH]),
            "wqt": wqt_by_f[fi],
            "scale": np.ascontiguousarray(scale_f[osl]),
            "zero": np.ascontiguousarray(zero_f[osl]),
            "bias": np.ascontiguousarray(bias_f[osl]),
            "lora_at": lora_at,
            "lora_b": np.ascontiguousarray(lora_B[osl]),
        })

    trace = bool(os.environ.get("BASS_KERNEL_TRACE"))
    res = run_bass_kernel_spmd(
        nc, in_maps, core_ids=list(range(T_SH * F_SH)), trace=trace,
    )
    if trace:
        _NC_CACHE["last_exec_time_ns"] = res.exec_time_ns
        _NC_CACHE["last_results"] = res

    y = np.empty((N_TOK, O), dtype=np.float32)
    for core in range(T_SH * F_SH):
        ti, fi = core % T_SH, core // T_SH
        y[ti * N_SH:(ti + 1) * N_SH, fi * O_SH:(fi + 1) * O_SH] = \
            res.results[core]["y"].T
    return y.reshape(B, S, O)


# revision 11
# speedup vs baseline: 1.8901x; 1.0765x over previous
"""Trainium2 Bass kernel for quantized-linear + LoRA (nn_LoRALinear).

Computes, for x:(4,2048,4096) f32, weight_quant:(4096,4096) i32 in [0,16),
scale/zero:(4096,1) f32, lora_A:(16,4096), lora_B:(4096,16), bias:(4096,):

    W = (weight_quant - zero) * scale
    y = x @ W.T + bias + 2.0 * (x @ lora_A.T) @ lora_B.T

Sharding across 8 NeuronCores: 4-way over tokens x 2-way over out-features.
Per core: x-slice (2048, 4096), weight rows slice (2048 of 4096), output
block (2048 tokens, 2048 features); host slices/permutes inputs and
stitches/transposes output blocks (layout only, no arithmetic).

Device algorithm (per core):

    P[o,n]   = sum_d (wq[o,d]-8) * x[n,d]          (PE; d-chunks 0..23 as
                                                    fp8e4 DoubleRow pairs,
                                                    chunks 24..31 in bf16)
             + sum_r B2[o,r] * t[r,n]              (K=18 fp32r matmul into the
             + (8-zero[o]) * rowsum[n]              same psum accumulation)
    y[o,n]   = scale[o] * P[o,n] + bias[o]         (ScalarE psum eviction)

with t = lora_A @ x.T (bf16) augmented by a ones-row giving rowsum,
B2 = 2*lora_B/scale. x and wq arrive host-pre-transposed ([d, n] f32 and
[d, o] uint8); gpsimd cast-DMAs land them directly in SBUF as bf16, the
DVE centers wq to fp8 and narrows x chunks 0..23 to fp8. Output stays
transposed [o, n]; the host de-transposes while stitching.
"""
import os
import sys
import types

sys.path.insert(0, "/opt/trn_rl_repo")

import numpy as np

import concourse.bass as bass
import concourse.mybir as mybir
import concourse.tile as tile
from concourse import bacc
from concourse.bass_utils import run_bass_kernel_spmd
from concourse.masks import make_identity

F32 = mybir.dt.float32
F32R = mybir.dt.float32r
BF16 = mybir.dt.bfloat16
FP8 = mybir.dt.float8e4
U8 = mybir.dt.uint8

DR = mybir.MatmulPerfMode.DoubleRow

# Problem shape (hardcoded per contract)
B, S, D, O, R = 4, 2048, 4096, 4096, 16
SCALING = 32.0 / 16.0
N_TOK = B * S            # 8192 tokens
T_SH, F_SH = 4, 2        # token shards x feature shards = 8 cores
N_SH = N_TOK // T_SH     # 2048 tokens per core
O_SH = O // F_SH         # 2048 out-features per core

NT = 4                   # n tiles per core
N_TILE = N_SH // NT      # 512
KC = D // 128            # 32 contraction chunks of 128
KC8 = 24                 # chunks done in fp8 DoubleRow (pairs 0..11)
KD8 = KC8 // 2           # 12 DoubleRow chunk-pairs
KCB = KC - KC8           # 8 chunks done in bf16 (accuracy headroom)
OT = O_SH // 128         # 16 o tiles
OQ = 4                   # o tiles per psum pass
WQ_CENTER = 8.0          # center wq (exact in fp8e4; small dot magnitude)


def _ensure_ntff_hook():
    """Best-effort: register the axon NTFF profile hook so trace=True works."""
    try:
        import antenv
        if "antenv.axon_hooks" not in sys.modules:
            hooks_mod = types.ModuleType("antenv.axon_hooks")
            hooks_mod._hook = None
            hooks_mod.set_axon_ntff_profile_hook = lambda h: setattr(hooks_mod, "_hook", h)
            hooks_mod.get_axon_ntff_profile_hook = lambda: hooks_mod._hook
            sys.modules["antenv.axon_hooks"] = hooks_mod
            antenv.axon_hooks = hooks_mod
        from trn_agent_boot.trn_boot import _ntff_profile_via_ctypes
        sys.modules["antenv.axon_hooks"].set_axon_ntff_profile_hook(
            _ntff_profile_via_ctypes("/opt/axon/libaxon_pjrt.so")
        )
        import concourse.bass_utils as bu
        bu.upload_artifacts = lambda tmpdir: tmpdir
    except Exception:
        pass


def build_nc() -> bass.Bass:
    nc = bacc.Bacc("TRN2", target_bir_lowering=False, debug=False)

    xt_d = nc.dram_tensor("xt", (D, N_SH), F32, kind="ExternalInput")
    wqt_d = nc.dram_tensor("wqt", (D, O_SH), U8, kind="ExternalInput")
    scale_d = nc.dram_tensor("scale", (O_SH,), F32, kind="ExternalInput")
    zero_d = nc.dram_tensor("zero", (O_SH,), F32, kind="ExternalInput")
    bias_d = nc.dram_tensor("bias", (O_SH,), F32, kind="ExternalInput")
    a_d = nc.dram_tensor("lora_at", (D, R), F32, kind="ExternalInput")
    b_d = nc.dram_tensor("lora_b", (O_SH, R), F32, kind="ExternalInput")
    y_d = nc.dram_tensor("y", (O_SH, N_SH), F32, kind="ExternalOutput")

    with tile.TileContext(nc) as tc:
        with (
            tc.tile_pool(name="const", bufs=1) as cpool,
            tc.tile_pool(name="wt", bufs=1) as wtpool,
            tc.tile_pool(name="xt", bufs=2) as xtpool,
            tc.tile_pool(name="xt8", bufs=2) as xt8pool,
            tc.tile_pool(name="stage", bufs=2) as stage,
            tc.tile_pool(name="tsb", bufs=2) as tsbpool,
            tc.tile_pool(name="outp", bufs=3) as outp,
            tc.tile_pool(name="ps_small", bufs=2, space="PSUM") as ps_small,
            tc.tile_pool(name="ps_t", bufs=1, space="PSUM") as ps_tp,
            tc.tile_pool(name="ps_acc", bufs=4, space="PSUM") as ps_accp,
        ):
            # ---------------- constants (small DMAs on the scalar queue) ----
            ident_f = cpool.tile([128, 128], F32)
            make_identity(nc, ident_f)
            ident_r = cpool.tile([128, 128], F32R)
            nc.vector.tensor_copy(ident_r[:], ident_f[:])

            # scale/bias/zero as [128 partitions, 16 o-tiles] f32
            scale_sb = cpool.tile([128, OT], F32)
            bias_sb = cpool.tile([128, OT], F32)
            zero_sb = cpool.tile([128, OT], F32)
            nc.scalar.dma_start(scale_sb[:], scale_d.rearrange("(t p) -> p t", p=128))
            nc.scalar.dma_start(bias_sb[:], bias_d.rearrange("(t p) -> p t", p=128))
            nc.scalar.dma_start(zero_sb[:], zero_d.rearrange("(t p) -> p t", p=128))
            rcp_sb = cpool.tile([128, OT], F32)
            nc.vector.reciprocal(rcp_sb[:], scale_sb[:])
            rcp2_sb = cpool.tile([128, OT], F32)
            nc.vector.tensor_scalar_mul(rcp2_sb[:], rcp_sb[:], float(SCALING))

            # B2augT [18, OT, 128] fp32r: rows 0..15 = (2*B/scale).T,
            # row 16 = (WQ_CENTER - zero)  [pairs with rowsum row of t_aug],
            # row 17 = bias/scale          [pairs with the ones row of t_sb]
            b2augT = cpool.tile([18, OT, 128], F32R)
            for t in range(OT):
                bblk = stage.tile([128, R], F32, tag="bblk")
                nc.scalar.dma_start(bblk[:], b_d[t * 128:(t + 1) * 128, :])
                pre = stage.tile([128, 18], F32R, tag="pre")
                nc.vector.tensor_scalar(
                    out=pre[:, 0:R], in0=bblk[:],
                    scalar1=rcp2_sb[:, t:t + 1], scalar2=None,
                    op0=mybir.AluOpType.mult,
                )
                nc.vector.tensor_scalar(
                    out=pre[:, R:R + 1], in0=zero_sb[:, t:t + 1],
                    scalar1=-1.0, scalar2=float(WQ_CENTER),
                    op0=mybir.AluOpType.mult, op1=mybir.AluOpType.add,
                )
                nc.vector.tensor_mul(
                    pre[:, R + 1:R + 2], bias_sb[:, t:t + 1], rcp_sb[:, t:t + 1]
                )
                psb = ps_small.tile([18, 128], F32R, tag="ps_sm")
                nc.tensor.transpose(psb[:], pre[:], ident_r[:])
                nc.vector.tensor_copy(b2augT[:, t, :], psb[:].bitcast(F32))

            # A_augT [128, KC, 17] bf16: cols 0..15 = A.T chunk, col16 = ones.
            # A.T arrives host-pre-transposed; one 3D cast-DMA + DVE copy.
            a_augT = cpool.tile([128, KC, R + 1], BF16)
            nc.gpsimd.memset(a_augT[:, :, R:R + 1], 1.0)
            ones32 = cpool.tile([32, N_TILE], F32)
            nc.gpsimd.memset(ones32[:], 1.0)
            a_stg = cpool.tile([128, KC, R], BF16)
            nc.gpsimd.dma_start(
                a_stg[:], a_d.rearrange("(k p) r -> p k r", p=128)
            )
            nc.vector.tensor_copy(a_augT[:, :, 0:R], a_stg[:])

            # ------- weights: resident, chunks 0..23 fp8 (centered), 24..31 bf16
            # wt8[p, k, ot, oi] = wqt[k*128+p, ot*128+oi] - 8      (k < 24)
            # wtb[p, k-24, ot, oi] = same, bf16                    (k >= 24)
            wt8 = wtpool.tile([128, KC8, OT, 128], FP8)
            wtb = wtpool.tile([128, KCB, OT, 128], BF16)

            def emit_og_build(og, k4s):
                # o-column group og covers o in [og*512, (og+1)*512)
                for k4 in k4s:
                    wst = stage.tile([128, 4, 512], BF16, tag="wst")
                    nc.gpsimd.dma_start(
                        wst[:],
                        wqt_d[k4 * 512:(k4 + 1) * 512,
                              og * 512:(og + 1) * 512]
                        .rearrange("(kk p) o -> p kk o", p=128),
                    )
                    if k4 < KC8 // 4:
                        dst = wt8[:, k4 * 4:(k4 + 1) * 4,
                                  og * OQ:(og + 1) * OQ, :]
                    else:
                        k4b = k4 - KC8 // 4
                        dst = wtb[:, k4b * 4:(k4b + 1) * 4,
                                  og * OQ:(og + 1) * OQ, :]
                    nc.vector.tensor_scalar(
                        out=dst, in0=wst[:],
                        scalar1=-WQ_CENTER, scalar2=None,
                        op0=mybir.AluOpType.add,
                    )

            # ---------------- main loop ----------------
            def emit_prep_data(nt):
                # xT bf16 [128, KC, N_TILE] straight from DRAM via cast-DMA,
                # quarter at a time; fp8 narrow for the DoubleRow chunks.
                xT = xtpool.tile([128, KC, N_TILE], BF16, tag="xT")
                xT8 = xt8pool.tile([128, KC8, N_TILE], FP8, tag="xT8")
                for kq in range(4):
                    nc.gpsimd.dma_start(
                        xT[:, kq * 8:(kq + 1) * 8, :],
                        xt_d[kq * 1024:(kq + 1) * 1024,
                             nt * N_TILE:(nt + 1) * N_TILE]
                        .rearrange("(k p) n -> p k n", p=128),
                    )
                    if kq < 3:
                        nc.vector.tensor_copy(
                            xT8[:, kq * 8:(kq + 1) * 8, :],
                            xT[:, kq * 8:(kq + 1) * 8, :],
                        )
                return xT, xT8

            def emit_prep_lora(xT):
                # t_aug [17, N_TILE] psum: rows 0..15 = A@x.T, row16 = rowsum
                ps_t = ps_tp.tile([R + 1, N_TILE], F32)
                for k in range(KC):
                    nc.tensor.matmul(
                        ps_t[:], a_augT[:, k, :], xT[:, k, :],
                        start=(k == 0), stop=(k == KC - 1),
                    )
                # t_sb rows 0..16 = t_aug, row 17 = 1.0 (ones base, overwrite)
                t_sb = tsbpool.tile([32, N_TILE], F32R, tag="t_sb")
                nc.vector.tensor_copy(t_sb[:], ones32[:])
                nc.vector.tensor_copy(t_sb[0:R + 1, :], ps_t[:])
                return t_sb

            def emit_oq_mains(nt, oq, xT, xT8):
                accs = []
                for _oi in range(OQ):
                    acc_tile = ps_accp.tile([128, N_TILE], F32, tag="acc")
                    accs.append(acc_tile)
                for kd in range(KD8):
                    for oi in range(OQ):
                        nc.tensor.matmul(
                            accs[oi][:],
                            wt8[:, 2 * kd:2 * kd + 2, oq * OQ + oi, :],
                            xT8[:, 2 * kd:2 * kd + 2, :],
                            start=(kd == 0), stop=False,
                            perf_mode=DR,
                        )
                for kb in range(KCB):
                    for oi in range(OQ):
                        nc.tensor.matmul(
                            accs[oi][:],
                            wtb[:, kb, oq * OQ + oi, :],
                            xT[:, KC8 + kb, :],
                            start=False, stop=False,
                        )
                return accs

            def emit_oq_tail(nt, oq, accs, t_sb):
                for oi in range(OQ):
                    ot = oq * OQ + oi
                    # lora + zero-correction + bias: K=18 fp32r matmul
                    nc.tensor.matmul(
                        accs[oi][:], b2augT[:, ot, :], t_sb[0:18, :],
                        start=False, stop=True,
                    )
                    # yT tile = scale[o]*P  (bias folded into the K=18 matmul)
                    yT_sb = outp.tile([128, N_TILE], F32, tag="yT")
                    nc.scalar.activation(
                        yT_sb[:], accs[oi][:],
                        mybir.ActivationFunctionType.Copy,
                        scale=scale_sb[:, ot:ot + 1],
                    )
                    # store transposed [o, n]; host de-transposes
                    nc.sync.dma_start(
                        y_d[ot * 128:(ot + 1) * 128,
                            nt * N_TILE:(nt + 1) * N_TILE],
                        yT_sb[:],
                    )

            # interleaved emission: weight/x cast-DMAs slotted so the gpsimd
            # queue feeds oq0's first matmuls as early as possible, and the
            # lora matmuls land between oq0's mains and its K=18 tail.
            xT0, xT8_0 = emit_prep_data(0)
            emit_og_build(0, range(KC // 4))
            a0 = emit_oq_mains(0, 0, xT0, xT8_0)
            t_sb0 = emit_prep_lora(xT0)
            emit_oq_tail(0, 0, a0, t_sb0)
            emit_og_build(1, range(KC // 4))
            a1 = emit_oq_mains(0, 1, xT0, xT8_0)
            emit_oq_tail(0, 1, a1, t_sb0)
            emit_og_build(2, range(KC // 4))
            a2 = emit_oq_mains(0, 2, xT0, xT8_0)
            emit_oq_tail(0, 2, a2, t_sb0)
            emit_og_build(3, range(KC // 4))
            a3 = emit_oq_mains(0, 3, xT0, xT8_0)
            emit_oq_tail(0, 3, a3, t_sb0)
            for nt in range(1, NT):
                xT, xT8 = emit_prep_data(nt)
                a0 = emit_oq_mains(nt, 0, xT, xT8)
                t_sb = emit_prep_lora(xT)
                emit_oq_tail(nt, 0, a0, t_sb)
                for oq in range(1, OQ):
                    aq = emit_oq_mains(nt, oq, xT, xT8)
                    emit_oq_tail(nt, oq, aq, t_sb)

    nc.finalize()
    return nc


_NC_CACHE: dict = {}


def _get_nc() -> bass.Bass:
    if "nc" not in _NC_CACHE:
        _ensure_ntff_hook()
        _NC_CACHE["nc"] = build_nc()
    return _NC_CACHE["nc"]


def kernel(x, weight_quant, scale, zero, lora_A, lora_B, bias):
    x = np.ascontiguousarray(np.asarray(x, dtype=np.float32)).reshape(N_TOK, D)
    weight_quant = np.asarray(weight_quant, dtype=np.int32)
    scale_f = np.asarray(scale, dtype=np.float32).reshape(O)
    zero_f = np.asarray(zero, dtype=np.float32).reshape(O)
    bias_f = np.asarray(bias, dtype=np.float32).reshape(O)
    lora_A = np.ascontiguousarray(np.asarray(lora_A, dtype=np.float32))
    lora_B = np.ascontiguousarray(np.asarray(lora_B, dtype=np.float32))

    nc = _get_nc()

    # host-side layout marshaling (transposes/slices/dtype repack only):
    # x.T per token shard, lora_A.T, and wq.T as uint8 per feature shard
    lora_at = np.ascontiguousarray(lora_A.T)
    xt_by_t = [
        np.ascontiguousarray(x[ti * N_SH:(ti + 1) * N_SH].T)
        for ti in range(T_SH)
    ]
    wqt_by_f = []
    for fi in range(F_SH):
        osl = slice(fi * O_SH, (fi + 1) * O_SH)
        wqt_by_f.append(
            np.ascontiguousarray(weight_quant[osl].T.astype(np.uint8))
        )

    in_maps = []
    for core in range(T_SH * F_SH):
        ti, fi = core % T_SH, core // T_SH
        osl = slice(fi * O_SH, (fi + 1) * O_SH)
        in_maps.append({
            "xt": xt_by_t[ti],
            "wqt": wqt_by_f[fi],
            "scale": np.ascontiguousarray(scale_f[osl]),
            "zero": np.ascontiguousarray(zero_f[osl]),
            "bias": np.ascontiguousarray(bias_f[osl]),
            "lora_at": lora_at,
            "lora_b": np.ascontiguousarray(lora_B[osl]),
        })

    trace = bool(os.environ.get("BASS_KERNEL_TRACE"))
    res = run_bass_kernel_spmd(
        nc, in_maps, core_ids=list(range(T_SH * F_SH)), trace=trace,
    )
    if trace:
        _NC_CACHE["last_exec_time_ns"] = res.exec_time_ns
        _NC_CACHE["last_results"] = res

    y = np.empty((N_TOK, O), dtype=np.float32)
    for core in range(T_SH * F_SH):
        ti, fi = core % T_SH, core // T_SH
        y[ti * N_SH:(ti + 1) * N_SH, fi * O_SH:(fi + 1) * O_SH] = \
            res.results[core]["y"].T
    return y.reshape(B, S, O)


# revision 16
# speedup vs baseline: 1.9596x; 1.0368x over previous
"""Trainium2 Bass kernel for quantized-linear + LoRA (nn_LoRALinear).

Computes, for x:(4,2048,4096) f32, weight_quant:(4096,4096) i32 in [0,16),
scale/zero:(4096,1) f32, lora_A:(16,4096), lora_B:(4096,16), bias:(4096,):

    W = (weight_quant - zero) * scale
    y = x @ W.T + bias + 2.0 * (x @ lora_A.T) @ lora_B.T

Sharding across 8 NeuronCores: 4-way over tokens x 2-way over out-features.
Per core: x-slice (2048, 4096), weight rows slice (2048 of 4096), output
block (2048 tokens, 2048 features); host slices/permutes inputs and
stitches/transposes output blocks (layout only, no arithmetic).

Device algorithm (per core):

    P[o,n]   = sum_d (wq[o,d]-8) * x[n,d]          (PE; d-chunks 0..23 as
                                                    fp8e4 DoubleRow pairs,
                                                    chunks 24..31 in bf16)
             + sum_r B2[o,r] * t[r,n]              (K=18 fp32r matmul into the
             + (8-zero[o]) * rowsum[n]              same psum accumulation)
    y[o,n]   = scale[o] * P[o,n] + bias[o]         (ScalarE psum eviction)

with t = lora_A @ x.T (bf16) augmented by a ones-row giving rowsum,
B2 = 2*lora_B/scale. x and wq arrive host-pre-transposed ([d, n] f32 and
[d, o] uint8); gpsimd cast-DMAs land them directly in SBUF as bf16, the
DVE centers wq to fp8 and narrows x chunks 0..23 to fp8. Output stays
transposed [o, n]; the host de-transposes while stitching.
"""
import os
import sys
import types

sys.path.insert(0, "/opt/trn_rl_repo")

import numpy as np

import concourse.bass as bass
import concourse.mybir as mybir
import concourse.tile as tile
from concourse import bacc
from concourse.bass_utils import run_bass_kernel_spmd
from concourse.masks import make_identity

F32 = mybir.dt.float32
F32R = mybir.dt.float32r
BF16 = mybir.dt.bfloat16
FP8 = mybir.dt.float8e4
U8 = mybir.dt.uint8

DR = mybir.MatmulPerfMode.DoubleRow

# Problem shape (hardcoded per contract)
B, S, D, O, R = 4, 2048, 4096, 4096, 16
SCALING = 32.0 / 16.0
N_TOK = B * S            # 8192 tokens
T_SH, F_SH = 4, 2        # token shards x feature shards = 8 cores
N_SH = N_TOK // T_SH     # 2048 tokens per core
O_SH = O // F_SH         # 2048 out-features per core

NT = 4                   # n tiles per core
N_TILE = N_SH // NT      # 512
KC = D // 128            # 32 contraction chunks of 128
KC8 = 24                 # chunks done in fp8 DoubleRow (pairs 0..11)
KD8 = KC8 // 2           # 12 DoubleRow chunk-pairs
KCB = KC - KC8           # 8 chunks done in bf16 (accuracy headroom)
OT = O_SH // 128         # 16 o tiles
OQ = 4                   # o tiles per psum pass
WQ_CENTER = 8.0          # center wq (exact in fp8e4; small dot magnitude)


def _ensure_ntff_hook():
    """Best-effort: register the axon NTFF profile hook so trace=True works."""
    try:
        import antenv
        if "antenv.axon_hooks" not in sys.modules:
            hooks_mod = types.ModuleType("antenv.axon_hooks")
            hooks_mod._hook = None
            hooks_mod.set_axon_ntff_profile_hook = lambda h: setattr(hooks_mod, "_hook", h)
            hooks_mod.get_axon_ntff_profile_hook = lambda: hooks_mod._hook
            sys.modules["antenv.axon_hooks"] = hooks_mod
            antenv.axon_hooks = hooks_mod
        from trn_agent_boot.trn_boot import _ntff_profile_via_ctypes
        sys.modules["antenv.axon_hooks"].set_axon_ntff_profile_hook(
            _ntff_profile_via_ctypes("/opt/axon/libaxon_pjrt.so")
        )
        import concourse.bass_utils as bu
        bu.upload_artifacts = lambda tmpdir: tmpdir
    except Exception:
        pass


def build_nc() -> bass.Bass:
    nc = bacc.Bacc("TRN2", target_bir_lowering=False, debug=False)

    xt_d = nc.dram_tensor("xt", (D, N_SH), F32, kind="ExternalInput")
    wqt_d = nc.dram_tensor("wqt", (D, O_SH), U8, kind="ExternalInput")
    scale_d = nc.dram_tensor("scale", (O_SH,), F32, kind="ExternalInput")
    zero_d = nc.dram_tensor("zero", (O_SH,), F32, kind="ExternalInput")
    bias_d = nc.dram_tensor("bias", (O_SH,), F32, kind="ExternalInput")
    a_d = nc.dram_tensor("lora_at", (D, R), F32, kind="ExternalInput")
    b_d = nc.dram_tensor("lora_b", (O_SH, R), F32, kind="ExternalInput")
    y_d = nc.dram_tensor("y", (O_SH, N_SH), F32, kind="ExternalOutput")

    with tile.TileContext(nc) as tc:
        with (
            tc.tile_pool(name="const", bufs=1) as cpool,
            tc.tile_pool(name="wt", bufs=1) as wtpool,
            tc.tile_pool(name="xt", bufs=2) as xtpool,
            tc.tile_pool(name="xt8", bufs=2) as xt8pool,
            tc.tile_pool(name="stage", bufs=2) as stage,
            tc.tile_pool(name="tsb", bufs=2) as tsbpool,
            tc.tile_pool(name="outp", bufs=3) as outp,
            tc.tile_pool(name="ps_small", bufs=2, space="PSUM") as ps_small,
            tc.tile_pool(name="ps_t", bufs=1, space="PSUM") as ps_tp,
            tc.tile_pool(name="ps_acc", bufs=4, space="PSUM") as ps_accp,
        ):
            # ---------------- constants (small DMAs on the scalar queue) ----
            ident_f = cpool.tile([128, 128], F32)
            make_identity(nc, ident_f)
            ident_r = cpool.tile([128, 128], F32R)
            nc.vector.tensor_copy(ident_r[:], ident_f[:])

            # scale/bias/zero as [128 partitions, 16 o-tiles] f32
            scale_sb = cpool.tile([128, OT], F32)
            bias_sb = cpool.tile([128, OT], F32)
            zero_sb = cpool.tile([128, OT], F32)
            nc.scalar.dma_start(scale_sb[:], scale_d.rearrange("(t p) -> p t", p=128))
            nc.scalar.dma_start(bias_sb[:], bias_d.rearrange("(t p) -> p t", p=128))
            nc.scalar.dma_start(zero_sb[:], zero_d.rearrange("(t p) -> p t", p=128))
            rcp_sb = cpool.tile([128, OT], F32)
            nc.vector.reciprocal(rcp_sb[:], scale_sb[:])
            rcp2_sb = cpool.tile([128, OT], F32)
            nc.vector.tensor_scalar_mul(rcp2_sb[:], rcp_sb[:], float(SCALING))

            # B2augT [18, OT, 128] fp32r: rows 0..15 = (2*B/scale).T,
            # row 16 = (WQ_CENTER - zero)  [pairs with rowsum row of t_aug],
            # row 17 = bias/scale          [pairs with the ones row of t_sb]
            b2augT = cpool.tile([18, OT, 128], F32R)

            def emit_b2_build():
                for t in range(OT):
                    bblk = stage.tile([128, R], F32, tag="bblk")
                    nc.scalar.dma_start(bblk[:], b_d[t * 128:(t + 1) * 128, :])
                    pre = stage.tile([128, 18], F32R, tag="pre")
                    nc.vector.tensor_scalar(
                        out=pre[:, 0:R], in0=bblk[:],
                        scalar1=rcp2_sb[:, t:t + 1], scalar2=None,
                        op0=mybir.AluOpType.mult,
                    )
                    nc.vector.tensor_scalar(
                        out=pre[:, R:R + 1], in0=zero_sb[:, t:t + 1],
                        scalar1=-1.0, scalar2=float(WQ_CENTER),
                        op0=mybir.AluOpType.mult, op1=mybir.AluOpType.add,
                    )
                    nc.vector.tensor_mul(
                        pre[:, R + 1:R + 2], bias_sb[:, t:t + 1], rcp_sb[:, t:t + 1]
                    )
                    psb = ps_small.tile([18, 128], F32R, tag="ps_sm")
                    nc.tensor.transpose(psb[:], pre[:], ident_r[:])
                    nc.vector.tensor_copy(b2augT[:, t, :], psb[:].bitcast(F32))

            # A_augT [128, KC, 17] bf16: cols 0..15 = A.T chunk, col16 = ones.
            # A.T arrives host-pre-transposed; one 3D cast-DMA + DVE copy.
            a_augT = cpool.tile([128, KC, R + 1], BF16)
            nc.gpsimd.memset(a_augT[:, :, R:R + 1], 1.0)
            ones32 = cpool.tile([32, N_TILE], F32)
            nc.gpsimd.memset(ones32[:], 1.0)
            a_stg = cpool.tile([128, KC, R], BF16)
            nc.gpsimd.dma_start(
                a_stg[:], a_d.rearrange("(k p) r -> p k r", p=128)
            )
            nc.vector.tensor_copy(a_augT[:, :, 0:R], a_stg[:])

            # ------- weights: resident, chunks 0..23 fp8 (centered), 24..31 bf16
            # wt8[p, k, ot, oi] = wqt[k*128+p, ot*128+oi] - 8      (k < 24)
            # wtb[p, k-24, ot, oi] = same, bf16                    (k >= 24)
            wt8 = wtpool.tile([128, KC8, OT, 128], FP8)
            wtb = wtpool.tile([128, KCB, OT, 128], BF16)

            def emit_og_build(og, k4s):
                # o-column group og covers o in [og*512, (og+1)*512)
                # raw u8 load on the fast sync queue; DVE centers + converts
                for k4 in k4s:
                    wst = stage.tile([128, 4, 512], U8, tag="wst")
                    nc.sync.dma_start(
                        wst[:],
                        wqt_d[k4 * 512:(k4 + 1) * 512,
                              og * 512:(og + 1) * 512]
                        .rearrange("(kk p) o -> p kk o", p=128),
                    )
                    if k4 < KC8 // 4:
                        dst = wt8[:, k4 * 4:(k4 + 1) * 4,
                                  og * OQ:(og + 1) * OQ, :]
                    else:
                        k4b = k4 - KC8 // 4
                        dst = wtb[:, k4b * 4:(k4b + 1) * 4,
                                  og * OQ:(og + 1) * OQ, :]
                    nc.vector.tensor_scalar(
                        out=dst, in0=wst[:],
                        scalar1=-WQ_CENTER, scalar2=None,
                        op0=mybir.AluOpType.add,
                    )

            # ---------------- main loop ----------------
            def emit_prep_chunks(nt, xT, xT8, k0, k1):
                # load x chunks [k0, k1) via cast-DMA; fp8-narrow the
                # DoubleRow portion
                nc.gpsimd.dma_start(
                    xT[:, k0:k1, :],
                    xt_d[k0 * 128:k1 * 128,
                         nt * N_TILE:(nt + 1) * N_TILE]
                    .rearrange("(k p) n -> p k n", p=128),
                )
                if k0 < KC8:
                    ke = min(k1, KC8)
                    nc.vector.tensor_copy(
                        xT8[:, k0:ke, :], xT[:, k0:ke, :]
                    )

            def emit_prep_data(nt):
                # xT bf16 [128, KC, N_TILE] straight from DRAM via cast-DMA,
                # quarter at a time; fp8 narrow for the DoubleRow chunks.
                xT = xtpool.tile([128, KC, N_TILE], BF16, tag="xT")
                xT8 = xt8pool.tile([128, KC8, N_TILE], FP8, tag="xT8")
                for kq in range(4):
                    emit_prep_chunks(nt, xT, xT8, kq * 8, (kq + 1) * 8)
                return xT, xT8

            def emit_prep_lora(xT):
                # t_aug [17, N_TILE] psum: rows 0..15 = A@x.T, row16 = rowsum
                ps_t = ps_tp.tile([R + 1, N_TILE], F32)
                for k in range(KC):
                    nc.tensor.matmul(
                        ps_t[:], a_augT[:, k, :], xT[:, k, :],
                        start=(k == 0), stop=(k == KC - 1),
                    )
                # t_sb rows 0..16 = t_aug, row 17 = 1.0 (ones base, overwrite)
                t_sb = tsbpool.tile([32, N_TILE], F32R, tag="t_sb")
                nc.vector.tensor_copy(t_sb[:], ones32[:])
                nc.vector.tensor_copy(t_sb[0:R + 1, :], ps_t[:])
                return t_sb

            def emit_oq_mains(nt, oq, xT, xT8):
                accs = []
                for _oi in range(OQ):
                    acc_tile = ps_accp.tile([128, N_TILE], F32, tag="acc")
                    accs.append(acc_tile)
                for kd in range(KD8):
                    for oi in range(OQ):
                        nc.tensor.matmul(
                            accs[oi][:],
                            wt8[:, 2 * kd:2 * kd + 2, oq * OQ + oi, :],
                            xT8[:, 2 * kd:2 * kd + 2, :],
                            start=(kd == 0), stop=False,
                            perf_mode=DR,
                        )
                for kb in range(KCB):
                    for oi in range(OQ):
                        nc.tensor.matmul(
                            accs[oi][:],
                            wtb[:, kb, oq * OQ + oi, :],
                            xT[:, KC8 + kb, :],
                            start=False, stop=False,
                        )
                return accs

            def emit_oq_tail(nt, oq, accs, t_sb):
                for oi in range(OQ):
                    ot = oq * OQ + oi
                    # lora + zero-correction + bias: K=18 fp32r matmul
                    nc.tensor.matmul(
                        accs[oi][:], b2augT[:, ot, :], t_sb[0:18, :],
                        start=False, stop=True,
                    )
                    # yT tile = scale[o]*P  (bias folded into the K=18 matmul)
                    yT_sb = outp.tile([128, N_TILE], F32, tag="yT")
                    nc.scalar.activation(
                        yT_sb[:], accs[oi][:],
                        mybir.ActivationFunctionType.Copy,
                        scale=scale_sb[:, ot:ot + 1],
                    )
                    # store transposed [o, n]; host de-transposes
                    nc.sync.dma_start(
                        y_d[ot * 128:(ot + 1) * 128,
                            nt * N_TILE:(nt + 1) * N_TILE],
                        yT_sb[:],
                    )

            # interleaved emission: weight/x cast-DMAs slotted so the queues
            # feed oq0's first matmuls as early as possible; the const build
            # and lora matmuls land between oq0's mains and its K=18 tail.
            xT0 = xtpool.tile([128, KC, N_TILE], BF16, tag="xT")
            xT8_0 = xt8pool.tile([128, KC8, N_TILE], FP8, tag="xT8")
            emit_prep_chunks(0, xT0, xT8_0, 0, 4)
            emit_og_build(0, [0, 1])
            emit_prep_chunks(0, xT0, xT8_0, 4, 8)
            emit_og_build(0, [2, 3])
            emit_prep_chunks(0, xT0, xT8_0, 8, 16)
            emit_og_build(0, [4, 5])
            emit_prep_chunks(0, xT0, xT8_0, 16, 24)
            emit_og_build(0, [6, 7])
            emit_prep_chunks(0, xT0, xT8_0, 24, 32)
            a0 = emit_oq_mains(0, 0, xT0, xT8_0)
            emit_b2_build()
            t_sb0 = emit_prep_lora(xT0)
            emit_oq_tail(0, 0, a0, t_sb0)
            emit_og_build(1, range(KC // 4))
            a1 = emit_oq_mains(0, 1, xT0, xT8_0)
            emit_oq_tail(0, 1, a1, t_sb0)
            emit_og_build(2, range(KC // 4))
            a2 = emit_oq_mains(0, 2, xT0, xT8_0)
            emit_oq_tail(0, 2, a2, t_sb0)
            emit_og_build(3, range(KC // 4))
            a3 = emit_oq_mains(0, 3, xT0, xT8_0)
            emit_oq_tail(0, 3, a3, t_sb0)
            for nt in range(1, NT):
                xT, xT8 = emit_prep_data(nt)
                a0 = emit_oq_mains(nt, 0, xT, xT8)
                t_sb = emit_prep_lora(xT)
                emit_oq_tail(nt, 0, a0, t_sb)
                for oq in range(1, OQ):
                    aq = emit_oq_mains(nt, oq, xT, xT8)
                    emit_oq_tail(nt, oq, aq, t_sb)

    nc.finalize()
    return nc


_NC_CACHE: dict = {}


def _get_nc() -> bass.Bass:
    if "nc" not in _NC_CACHE:
        _ensure_ntff_hook()
        _NC_CACHE["nc"] = build_nc()
    return _NC_CACHE["nc"]


def kernel(x, weight_quant, scale, zero, lora_A, lora_B, bias):
    x = np.ascontiguousarray(np.asarray(x, dtype=np.float32)).reshape(N_TOK, D)
    weight_quant = np.asarray(weight_quant, dtype=np.int32)
    scale_f = np.asarray(scale, dtype=np.float32).reshape(O)
    zero_f = np.asarray(zero, dtype=np.float32).reshape(O)
    bias_f = np.asarray(bias, dtype=np.float32).reshape(O)
    lora_A = np.ascontiguousarray(np.asarray(lora_A, dtype=np.float32))
    lora_B = np.ascontiguousarray(np.asarray(lora_B, dtype=np.float32))

    nc = _get_nc()

    # host-side layout marshaling (transposes/slices/dtype repack only):
    # x.T per token shard, lora_A.T, and wq.T as uint8 per feature shard
    lora_at = np.ascontiguousarray(lora_A.T)
    xt_by_t = [
        np.ascontiguousarray(x[ti * N_SH:(ti + 1) * N_SH].T)
        for ti in range(T_SH)
    ]
    wqt_by_f = []
    for fi in range(F_SH):
        osl = slice(fi * O_SH, (fi + 1) * O_SH)
        wqt_by_f.append(
            np.ascontiguousarray(weight_quant[osl].T.astype(np.uint8))
        )

    in_maps = []
    for core in range(T_SH * F_SH):
        ti, fi = core % T_SH, core // T_SH
        osl = slice(fi * O_SH, (fi + 1) * O_SH)
        in_maps.append({
            "xt": xt_by_t[ti],
            "wqt": wqt_by_f[fi],
            "scale": np.ascontiguousarray(scale_f[osl]),
            "zero": np.ascontiguousarray(zero_f[osl]),
            "bias": np.ascontiguousarray(bias_f[osl]),
            "lora_at": lora_at,
            "lora_b": np.ascontiguousarray(lora_B[osl]),
        })

    trace = bool(os.environ.get("BASS_KERNEL_TRACE"))
    res = run_bass_kernel_spmd(
        nc, in_maps, core_ids=list(range(T_SH * F_SH)), trace=trace,
    )
    if trace:
        _NC_CACHE["last_exec_time_ns"] = res.exec_time_ns
        _NC_CACHE["last_results"] = res

    y = np.empty((N_TOK, O), dtype=np.float32)
    for core in range(T_SH * F_SH):
        ti, fi = core % T_SH, core // T_SH
        y[ti * N_SH:(ti + 1) * N_SH, fi * O_SH:(fi + 1) * O_SH] = \
            res.results[core]["y"].T
    return y.reshape(B, S, O)
